# revision 3
# baseline (speedup 1.0000x reference)
"""Trainium2 Bass kernel for nn_Camada_33612414059004.

Computes, for x:[B,N,D,S], M:[N,N], w_syn:[N,D,S], b_dend:[N,D],
w_dend:[N,D], b_soma:[N]:

    xm    = einsum('bids,oi->bods', x, M)
    dend  = tanh(einsum('bnds,nds->bnd', xm, w_syn) + b_dend)
    soma  = einsum('bnd,nd->bn', dend, w_dend) + b_soma
    out   = sigmoid(soma)                                  # [B, N]

Sharding: data-parallel over batch across 8 NeuronCores (B=64 -> 8/core),
zero cross-core communication.

fp8 path: M is binary {0,1} (exact in e4m3) and x is ~N(0,1) (3.6% RMS
quantization error that the saturated tanh crushes to ~5e-3 end-to-end),
so both matmul operands are e4m3 and the PE runs in DoubleRow perf mode:
each matmul contracts 256 input neurons (2 k-chunks packed 2-per-cell)
at the same 216ns/512-col rate as bf16 -- a true 2x.  w_syn is also fp8
(the drain-multiply upconverts); remaining params bf16-scale fp32.

Layout: output neurons `o` on the 128 SBUF partitions (8 o-tiles), free
dim (b, d, s).  x and mt are host-packed into k-pair-major [128, 2048]
fp8 chunks (2KB/partition rows).  The DMA fabric sustains ~250-290GB/s
per core with all 8 cores streaming, so the ~2.2MB input stream is the
pacing item up to the last k-pair chunk.

Schedule: two accumulation waves of 4 o-tiles (PSUM fits half the
problem: each o-tile takes a [128,1024] 2-bank PSUM tile).  Wave 1 runs
kp-outer riding the DMA stream; each tile's fused [128,1024] PSUM
drain-multiply (DVE, bf16 product out) frees its banks for a wave-2
tile whose 8 matmuls then run kp-inner.  s-reduces run as GpSimd
pairwise trees (t0-6) writing into a collected dp buffer; bias+tanh+
soma+sigmoid for t0-6 are each ONE batched wide op (GpS/ACT/DVE); the
last tile t7 gets a private latency chain on DVE/ACT.  Output leaves on
the idle Sync HWDGE.
"""

import numpy as np
import ml_dtypes
from contextlib import ExitStack

import concourse.bass as bass
import concourse.mybir as mybir
import concourse.tile as tile

B, N, D, S = 64, 1024, 8, 16
NCORES = 8
BC = B // NCORES          # batches per core = 8
DS = D * S                # 128
P = 128                   # SBUF partitions
KT = N // P               # 8 contraction chunks (input neurons)
KP = KT // 2              # 4 k-pair chunks for DoubleRow
OT = N // P               # 8 output-neuron tiles
FH = 512                  # matmul moving free dim (one fp32 PSUM bank)
BD = BC * D               # 64
GRP = 4                   # o-tiles per accumulation wave (PSUM: 4x2 banks)
NWARM = 6                 # PE warm-up matmuls (HAM clock boost)
TB = OT - 1               # tiles in the batched tail path (t0..t6)

F32 = mybir.dt.float32
BF16 = mybir.dt.bfloat16
FP8 = mybir.dt.float8e4
NP_FP8 = ml_dtypes.float8_e4m3

_NC_CACHE = {}


def legalize_waits(nc, max_attached=1):
    """Split multi-semaphore waits onto preceding same-engine NOPs.

    The walrus build in this environment accepts at most one sync-wait
    command per instruction (setupSyncWait: "Too many sync wait commands"),
    but Tile attaches one wait per out-of-date engine clock.  An engine is
    in-order, so hoisting the extra waits onto NOPs immediately before the
    instruction is semantics-preserving.
    """
    nid = 0
    for f in nc.m.functions:
        for blk in f.blocks:
            new = []
            changed = False
            for inst in blk.instructions:
                si = inst.sync_info
                if si is not None and si.on_wait and len(si.on_wait) > max_attached:
                    waits = list(si.on_wait)
                    for w in waits[:-max_attached]:
                        nid += 1
                        nop = mybir.InstNoOp(name=f"WSPLIT-{nid}", ins=[], outs=[])
                        nop.engine = inst.engine
                        nop.sync_info = mybir.SyncInfo(on_wait=[w], on_update=[])
                        new.append(nop)
                    inst.sync_info = mybir.SyncInfo(
                        on_wait=waits[-max_attached:], on_update=list(si.on_update)
                    )
                    changed = True
                new.append(inst)
            if changed:
                blk.instructions = new
    return nc


def build_nc(legalize=True):
    """Build the single-core Bass program (SPMD: same program on all cores)."""
    nc = bass.Bass()
    mt = nc.declare_dram_parameter("mt", [KP * P, 2 * N], FP8, isOutput=False)
    xc = nc.declare_dram_parameter("xc", [KP * P, 2 * BC * DS], FP8, isOutput=False)
    wsyn = nc.declare_dram_parameter("wsyn", [P, OT * DS], FP8, isOutput=False)
    prm = nc.declare_dram_parameter("prm", [P, 2 * OT * D + OT], F32, isOutput=False)
    out = nc.declare_dram_parameter("out", [P, OT * BC], F32, isOutput=True)

    AF = mybir.ActivationFunctionType
    AX = mybir.AxisListType
    OP = mybir.AluOpType
    DR = mybir.MatmulPerfMode.DoubleRow
    B0, W1, B1 = 0, OT * D, 2 * OT * D    # col offsets in prm

    with tile.TileContext(nc) as tc, ExitStack() as ctx:
        wpool = ctx.enter_context(tc.tile_pool(name="weights", bufs=1))
        xpool = ctx.enter_context(tc.tile_pool(name="xin", bufs=1))
        pspool = ctx.enter_context(tc.tile_pool(name="ps", bufs=GRP, space="PSUM"))
        prpool = ctx.enter_context(tc.tile_pool(name="prp", bufs=3))
        smpool = ctx.enter_context(tc.tile_pool(name="smp", bufs=2))

        # --- PE pre-warm on Vector-memset scratch: sustains PE activity
        # through the DMA wait so the HAM 1.2->2.4GHz boost engages by the
        # time real matmuls stream. ---
        warm_sb = wpool.tile([P, FH], BF16, tag="warm", name="warm_sb")
        nc.vector.memset(warm_sb[:], 0.0)
        warm_ps = pspool.tile([P, 2 * FH], F32, tag="ps", name="warm_ps")
        for _ in range(NWARM):
            nc.tensor.matmul(
                warm_ps[:, 0:FH], lhsT=warm_sb[:, 0:P], rhs=warm_sb[:],
                start=True, stop=True,
            )

        # --- input DMAs: x k-pair chunks on Sync, mt chunks on Scalar
        # (parallel HWDGE issue); [128, 2KB-row] fp8 chunks. ---
        x_tiles, mt_tiles = [], []
        x0_dma = None
        for kp in range(KP):
            xt = xpool.tile([P, 2 * BC * DS], FP8, tag=f"x{kp}", name=f"x{kp}")
            mtk = xpool.tile([P, 2 * N], FP8, tag=f"m{kp}", name=f"m{kp}")
            xdma = nc.sync.dma_start(xt[:], xc[kp * P:(kp + 1) * P, :])
            if kp == 0:
                x0_dma = xdma
            nc.scalar.dma_start(mtk[:], mt[kp * P:(kp + 1) * P, :])
            x_tiles.append(xt)
            mt_tiles.append(mtk)

        # Per-neuron parameters ride behind the first x chunk (needed only
        # once the first accumulation chain completes).
        wsyn_sb = wpool.tile([P, OT * DS], FP8, tag="wsyn", name="wsyn_sb")
        prm_sb = wpool.tile([P, 2 * OT * D + OT], F32, tag="prm", name="prm_sb")
        wdma = nc.gpsimd.dma_start(wsyn_sb[:], wsyn[:, :])
        nc.gpsimd.dma_start(prm_sb[:], prm[:, :])
        from bass_rust import add_dep_helper
        add_dep_helper(wdma.ins, x0_dma.ins, sync=True,
                       reason="params after critical first chunk")

        out_sb = wpool.tile([P, OT * BC], F32, tag="out", name="out_sb")
        # Collected dendrite pre-activations for the batched t0-6 tail:
        # col = (t, b, d).
        dp_all = wpool.tile([P, TB * BD], F32, tag="dpall", name="dp_all")

        def dr_mm(ps_t, t, kp, h):
            # DoubleRow fp8 matmul: contracts k-chunks 2*kp and 2*kp+1 at
            # once (two weights per PE cell); 3D APs [128, 2, free].
            nc.tensor.matmul(
                ps_t[:, h * FH:(h + 1) * FH],
                lhsT=mt_tiles[kp][:].rearrange("p (r o) -> p r o", r=2)
                [:, :, t * P:(t + 1) * P],
                rhs=x_tiles[kp][:].rearrange("p (r n) -> p r n", r=2)
                [:, :, h * FH:(h + 1) * FH],
                start=(kp == 0),
                stop=(kp == KP - 1),
                perf_mode=DR,
            )

        def drain(t, ps_t):
            # Fused 2-bank PSUM drain: prod[o,(b,ds)] = xm * w_syn
            # (broadcast over the 8 (h,b) groups), bf16 product out.
            prod = prpool.tile([P, 2 * FH], BF16, tag="prod", name=f"prod{t}")
            nc.vector.tensor_mul(
                prod[:].rearrange("p (b q) -> p b q", b=BC),
                ps_t[:].rearrange("p (b q) -> p b q", b=BC),
                wsyn_sb[:, t * DS:(t + 1) * DS].unsqueeze(1)
                .broadcast_to([P, BC, DS]),
            )
            return prod

        def gps_tree(t, prod):
            # s-reduce as a GpSimd pairwise tree (bf16), final level lands
            # fp32 in dp_all's (t, b, d) slot.
            pv = prod[:].rearrange("p (bd s) -> p bd s", s=S)
            gr1 = smpool.tile([P, BD * 8], BF16, tag="gr1", name=f"gr1{t}")
            nc.gpsimd.tensor_add(
                gr1[:].rearrange("p (bd s) -> p bd s", s=8),
                pv[:, :, 0:8], pv[:, :, 8:16],
            )
            g1v = gr1[:].rearrange("p (bd s) -> p bd s", s=8)
            gr2 = smpool.tile([P, BD * 4], BF16, tag="gr2", name=f"gr2{t}")
            nc.gpsimd.tensor_add(
                gr2[:].rearrange("p (bd s) -> p bd s", s=4),
                g1v[:, :, 0:4], g1v[:, :, 4:8],
            )
            g2v = gr2[:].rearrange("p (bd s) -> p bd s", s=4)
            gr3 = smpool.tile([P, BD * 2], BF16, tag="gr3", name=f"gr3{t}")
            nc.gpsimd.tensor_add(
                gr3[:].rearrange("p (bd s) -> p bd s", s=2),
                g2v[:, :, 0:2], g2v[:, :, 2:4],
            )
            g3v = gr3[:].rearrange("p (bd s) -> p bd s", s=2)
            nc.gpsimd.tensor_add(
                dp_all[:, t * BD:(t + 1) * BD].unsqueeze(2),
                g3v[:, :, 0:1], g3v[:, :, 1:2],
            )

        # --- Wave 1: o-tiles 0..3, kp-outer (paces with the DMA stream);
        # each tile's drain follows its last matmul, freeing 2 banks. ---
        pst = {}
        for t in range(GRP):
            pst[t] = pspool.tile([P, 2 * FH], F32, tag="ps", name=f"ps{t}")
        for kp in range(KP):
            for t in range(GRP):
                for h in range(2):
                    dr_mm(pst[t], t, kp, h)
        prods = {}
        for t in range(GRP):
            prods[t] = drain(t, pst[t])
            gps_tree(t, prods[t])

        # --- Wave 2: o-tiles 4..7 kp-inner, each claiming banks freed by
        # the corresponding wave-1 drain; drain ASAP after the 8th MM. ---
        for t in range(GRP, OT):
            ps_t = pspool.tile([P, 2 * FH], F32, tag="ps", name=f"ps{t}")
            for h in range(2):
                for kp in range(KP):
                    dr_mm(ps_t, t, kp, h)
            prods[t] = drain(t, ps_t)
            if t < OT - 1:
                gps_tree(t, prods[t])

        # --- t7 private latency chain (DVE + ACT) -> out_sb[:, 56:64]. ---
        t7 = OT - 1
        dp7 = smpool.tile([P, BD], F32, tag="dp7", name="dp7")
        nc.vector.tensor_reduce(
            dp7[:], prods[t7][:].rearrange("p (bd s) -> p bd s", s=S),
            axis=AX.X, op=OP.add,
        )
        nc.vector.tensor_add(
            dp7[:].rearrange("p (b d) -> p b d", d=D),
            dp7[:].rearrange("p (b d) -> p b d", d=D),
            prm_sb[:, B0 + t7 * D:B0 + (t7 + 1) * D].unsqueeze(1)
            .broadcast_to([P, BC, D]),
        )
        dend7 = smpool.tile([P, BD], F32, tag="dend7", name="dend7")
        nc.scalar.activation(dend7[:], dp7[:], AF.Tanh)
        sp7 = smpool.tile([P, BD], F32, tag="sp7", name="sp7")
        nc.vector.tensor_mul(
            sp7[:].rearrange("p (b d) -> p b d", d=D),
            dend7[:].rearrange("p (b d) -> p b d", d=D),
            prm_sb[:, W1 + t7 * D:W1 + (t7 + 1) * D].unsqueeze(1)
            .broadcast_to([P, BC, D]),
        )
        soma7 = smpool.tile([P, BC], F32, tag="soma7", name="soma7")
        nc.vector.tensor_reduce(
            soma7[:], sp7[:].rearrange("p (b d) -> p b d", d=D),
            axis=AX.X, op=OP.add,
        )
        nc.scalar.activation(
            out_sb[:, t7 * BC:(t7 + 1) * BC], soma7[:], AF.Sigmoid,
            bias=prm_sb[:, B1 + t7:B1 + t7 + 1],
        )

        # --- Batched t0-6 tail: one wide op per stage. ---
        # dp_all[p, (t,b,d)] += b_dend[p, (t,d)]  (broadcast over b)
        nc.gpsimd.tensor_add(
            dp_all[:].rearrange("p (t b d) -> p t b d", t=TB, b=BC),
            dp_all[:].rearrange("p (t b d) -> p t b d", t=TB, b=BC),
            prm_sb[:, B0:B0 + TB * D]
            .rearrange("p (t d) -> p t d", t=TB).unsqueeze(2)
            .broadcast_to([P, TB, BC, D]),
        )
        dend_all = wpool.tile([P, TB * BD], F32, tag="dendall", name="dend_all")
        nc.scalar.activation(dend_all[:], dp_all[:], AF.Tanh)
        sp_all = wpool.tile([P, TB * BD], F32, tag="spall", name="sp_all")
        nc.vector.tensor_mul(
            sp_all[:].rearrange("p (t b d) -> p t b d", t=TB, b=BC),
            dend_all[:].rearrange("p (t b d) -> p t b d", t=TB, b=BC),
            prm_sb[:, W1:W1 + TB * D]
            .rearrange("p (t d) -> p t d", t=TB).unsqueeze(2)
            .broadcast_to([P, TB, BC, D]),
        )
        soma_all = wpool.tile([P, TB * BC], F32, tag="somaall", name="soma_all")
        nc.vector.tensor_reduce(
            soma_all[:], sp_all[:].rearrange("p (tb d) -> p tb d", d=D),
            axis=AX.X, op=OP.add,
        )
        nc.vector.tensor_add(
            soma_all[:].rearrange("p (t b) -> p t b", t=TB),
            soma_all[:].rearrange("p (t b) -> p t b", t=TB),
            prm_sb[:, B1:B1 + TB].unsqueeze(2).broadcast_to([P, TB, BC]),
        )
        nc.scalar.activation(out_sb[:, 0:TB * BC], soma_all[:], AF.Sigmoid)

        # Output on the (input-idle-by-now) Sync HWDGE.
        nc.sync.dma_start(out[:, :], out_sb[:])

    if legalize:
        legalize_waits(nc)
    return nc


def get_nc():
    key = "fp8dr2"
    if key not in _NC_CACHE:
        _NC_CACHE[key] = build_nc()
    return _NC_CACHE[key]


def pack_kpairs(a):
    """[N, C] -> [KP*P, 2*C], row kp*128+p holding [r=0 | r=1] halves of
    the k-pair (source row i = kp*256 + r*128 + p)."""
    C = a.shape[1]
    return np.ascontiguousarray(
        a.reshape(KP, 2, P, C).transpose(0, 2, 1, 3).reshape(KP * P, 2 * C)
    )


def prepare_in_maps(x, matriz_conexao, w_syn, b_dend, w_dend, b_soma):
    x = np.asarray(x, dtype=np.float32)
    mt_np = pack_kpairs(
        np.ascontiguousarray(np.asarray(matriz_conexao, np.float32).T)
    ).astype(NP_FP8)
    ws = np.asarray(w_syn, np.float32).reshape(OT, P, DS).transpose(1, 0, 2) \
        .reshape(P, OT * DS).astype(NP_FP8)
    bd = np.asarray(b_dend, np.float32).reshape(OT, P, D).transpose(1, 0, 2).reshape(P, OT * D)
    wd = np.asarray(w_dend, np.float32).reshape(OT, P, D).transpose(1, 0, 2).reshape(P, OT * D)
    bs = np.asarray(b_soma, np.float32).reshape(OT, P).T
    prm_np = np.ascontiguousarray(
        np.concatenate([bd, wd, bs], axis=1).astype(np.float32))
    xt = x.transpose(1, 0, 2, 3).reshape(N, B, DS)
    in_maps = []
    for c in range(NCORES):
        xc_np = pack_kpairs(
            np.ascontiguousarray(
                xt[:, c * BC:(c + 1) * BC, :].reshape(N, BC * DS))
        ).astype(NP_FP8)
        in_maps.append({"mt": mt_np, "xc": xc_np,
                        "wsyn": np.ascontiguousarray(ws), "prm": prm_np})
    return in_maps


def assemble_output(results):
    outs = []
    for c in range(NCORES):
        oc = np.asarray(results[c]["out"])          # [P, OT*BC] = (oi, (t, b))
        outs.append(oc.reshape(P, OT, BC).transpose(2, 1, 0).reshape(BC, N))
    return np.ascontiguousarray(np.concatenate(outs, axis=0).astype(np.float32))


def kernel(x, matriz_conexao, w_syn, b_dend, w_dend, b_soma):
    from concourse.bass_utils import run_bass_kernel_spmd
    in_maps = prepare_in_maps(x, matriz_conexao, w_syn, b_dend, w_dend, b_soma)
    nc = get_nc()
    res = run_bass_kernel_spmd(nc, in_maps, list(range(NCORES)))
    return assemble_output(res.results)


# revision 7
# speedup vs baseline: 1.3661x; 1.3661x over previous
"""Trainium2 Bass kernel for nn_Camada_33612414059004.

Computes, for x:[B,N,D,S], M:[N,N], w_syn:[N,D,S], b_dend:[N,D],
w_dend:[N,D], b_soma:[N]:

    xm    = einsum('bids,oi->bods', x, M)
    dend  = tanh(einsum('bnds,nds->bnd', xm, w_syn) + b_dend)
    soma  = einsum('bnd,nd->bn', dend, w_dend) + b_soma
    out   = sigmoid(soma)                                  # [B, N]

Sharding: data-parallel over batch across 8 NeuronCores (B=64 -> 8/core),
zero cross-core communication.

fp8 path: M is binary {0,1} (exact in e4m3) and x is ~N(0,1) (3.6% RMS
quantization error that the saturated tanh crushes to ~5e-3 end-to-end),
so both matmul operands are e4m3 and the PE runs in DoubleRow perf mode:
each matmul contracts 256 input neurons (2 k-chunks packed 2-per-cell)
at the same 216ns/512-col rate as bf16 -- a true 2x.  w_syn is also fp8
(the drain-multiply upconverts); remaining params bf16-scale fp32.

Layout: output neurons `o` on the 128 SBUF partitions (8 o-tiles), free
dim (b, d, s).  x and mt are host-packed into k-pair-major [128, 2048]
fp8 chunks (2KB/partition rows).  The DMA fabric sustains ~250-290GB/s
per core with all 8 cores streaming, so the ~2.2MB input stream is the
pacing item up to the last k-pair chunk.

Schedule: two accumulation waves of 4 o-tiles (PSUM fits half the
problem: each o-tile takes a [128,1024] 2-bank PSUM tile).  Wave 1 runs
kp-outer riding the DMA stream; each tile's fused [128,1024] PSUM
drain-multiply (DVE, bf16 product out) frees its banks for a wave-2
tile whose 8 matmuls then run kp-inner.  s-reduces run as GpSimd
pairwise trees (t0-6) writing into a collected dp buffer; bias+tanh+
soma+sigmoid for t0-6 are each ONE batched wide op (GpS/ACT/DVE); the
last tile t7 gets a private latency chain on DVE/ACT.  Output leaves on
the idle Sync HWDGE.
"""

import numpy as np
import ml_dtypes
from contextlib import ExitStack

import concourse.bass as bass
import concourse.mybir as mybir
import concourse.tile as tile

B, N, D, S = 64, 1024, 8, 16
NCORES = 8
BC = B // NCORES          # batches per core = 8
DS = D * S                # 128
P = 128                   # SBUF partitions
KT = N // P               # 8 contraction chunks (input neurons)
KP = KT // 2              # 4 k-pair chunks for DoubleRow
OT = N // P               # 8 output-neuron tiles
FH = 512                  # matmul moving free dim (one fp32 PSUM bank)
BD = BC * D               # 64
GRP = 4                   # o-tiles per accumulation wave (PSUM: 4x2 banks)
NWARM = 6                 # PE warm-up matmuls (HAM clock boost)
TB = OT - 1               # tiles in the batched tail path (t0..t6)

F32 = mybir.dt.float32
BF16 = mybir.dt.bfloat16
FP8 = mybir.dt.float8e4
NP_FP8 = ml_dtypes.float8_e4m3

_NC_CACHE = {}


def legalize_waits(nc, max_attached=1):
    """Split multi-semaphore waits onto preceding same-engine NOPs.

    The walrus build in this environment accepts at most one sync-wait
    command per instruction (setupSyncWait: "Too many sync wait commands"),
    but Tile attaches one wait per out-of-date engine clock.  An engine is
    in-order, so hoisting the extra waits onto NOPs immediately before the
    instruction is semantics-preserving.
    """
    nid = 0
    for f in nc.m.functions:
        for blk in f.blocks:
            new = []
            changed = False
            for inst in blk.instructions:
                si = inst.sync_info
                if si is not None and si.on_wait and len(si.on_wait) > max_attached:
                    waits = list(si.on_wait)
                    for w in waits[:-max_attached]:
                        nid += 1
                        nop = mybir.InstNoOp(name=f"WSPLIT-{nid}", ins=[], outs=[])
                        nop.engine = inst.engine
                        nop.sync_info = mybir.SyncInfo(on_wait=[w], on_update=[])
                        new.append(nop)
                    inst.sync_info = mybir.SyncInfo(
                        on_wait=waits[-max_attached:], on_update=list(si.on_update)
                    )
                    changed = True
                new.append(inst)
            if changed:
                blk.instructions = new
    return nc


def build_nc(legalize=True):
    """Build the single-core Bass program (SPMD: same program on all cores)."""
    nc = bass.Bass()
    mt = nc.declare_dram_parameter("mt", [KP * P, 2 * N], FP8, isOutput=False)
    xc = nc.declare_dram_parameter("xc", [KP * P, 2 * BC * DS], FP8, isOutput=False)
    wsyn = nc.declare_dram_parameter("wsyn", [P, OT * DS], FP8, isOutput=False)
    prm = nc.declare_dram_parameter("prm", [P, 2 * OT * D + OT], F32, isOutput=False)
    out = nc.declare_dram_parameter("out", [P, OT * BC], F32, isOutput=True)

    AF = mybir.ActivationFunctionType
    AX = mybir.AxisListType
    OP = mybir.AluOpType
    DR = mybir.MatmulPerfMode.DoubleRow
    B0, W1, B1 = 0, OT * D, 2 * OT * D    # col offsets in prm

    with tile.TileContext(nc) as tc, ExitStack() as ctx:
        wpool = ctx.enter_context(tc.tile_pool(name="weights", bufs=1))
        xpool = ctx.enter_context(tc.tile_pool(name="xin", bufs=1))
        pspool = ctx.enter_context(tc.tile_pool(name="ps", bufs=GRP, space="PSUM"))
        prpool = ctx.enter_context(tc.tile_pool(name="prp", bufs=8))
        smpool = ctx.enter_context(tc.tile_pool(name="smp", bufs=2))

        # --- PE pre-warm on Vector-memset scratch: sustains PE activity
        # through the DMA wait so the HAM 1.2->2.4GHz boost engages by the
        # time real matmuls stream. ---
        warm_sb = wpool.tile([P, FH], BF16, tag="warm", name="warm_sb")
        nc.vector.memset(warm_sb[:], 0.0)
        warm_ps = pspool.tile([P, 2 * FH], F32, tag="ps", name="warm_ps")
        for _ in range(NWARM):
            nc.tensor.matmul(
                warm_ps[:, 0:FH], lhsT=warm_sb[:, 0:P], rhs=warm_sb[:],
                start=True, stop=True,
            )

        # --- input DMAs: x k-pair chunks on Sync, mt chunks on Scalar
        # (parallel HWDGE issue); [128, 2KB-row] fp8 chunks. ---
        x_tiles, mt_tiles = [], []
        x0_dma = None
        for kp in range(KP):
            xt = xpool.tile([P, 2 * BC * DS], FP8, tag=f"x{kp}", name=f"x{kp}")
            mtk = xpool.tile([P, 2 * N], FP8, tag=f"m{kp}", name=f"m{kp}")
            xdma = nc.sync.dma_start(xt[:], xc[kp * P:(kp + 1) * P, :])
            if kp == 0:
                x0_dma = xdma
            nc.scalar.dma_start(mtk[:], mt[kp * P:(kp + 1) * P, :])
            x_tiles.append(xt)
            mt_tiles.append(mtk)

        # Per-neuron parameters ride behind the first x chunk (needed only
        # once the first accumulation chain completes).
        wsyn_sb = wpool.tile([P, OT * DS], FP8, tag="wsyn", name="wsyn_sb")
        prm_sb = wpool.tile([P, 2 * OT * D + OT], F32, tag="prm", name="prm_sb")
        wdma = nc.gpsimd.dma_start(wsyn_sb[:], wsyn[:, :])
        nc.gpsimd.dma_start(prm_sb[:], prm[:, :])
        from bass_rust import add_dep_helper
        add_dep_helper(wdma.ins, x0_dma.ins, sync=True,
                       reason="params after critical first chunk")

        out_sb = wpool.tile([P, OT * BC], F32, tag="out", name="out_sb")
        # Collected dendrite pre-activations for the batched t0-6 tail:
        # col = (t, b, d).
        dp_all = wpool.tile([P, TB * BD], F32, tag="dpall", name="dp_all")

        def dr_mm(ps_t, t, kp, h):
            # DoubleRow fp8 matmul: contracts k-chunks 2*kp and 2*kp+1 at
            # once (two weights per PE cell); 3D APs [128, 2, free].
            nc.tensor.matmul(
                ps_t[:, h * FH:(h + 1) * FH],
                lhsT=mt_tiles[kp][:].rearrange("p (r o) -> p r o", r=2)
                [:, :, t * P:(t + 1) * P],
                rhs=x_tiles[kp][:].rearrange("p (r n) -> p r n", r=2)
                [:, :, h * FH:(h + 1) * FH],
                start=(kp == 0),
                stop=(kp == KP - 1),
                perf_mode=DR,
            )

        def drain(t, ps_t):
            # Fused 2-bank PSUM drain: prod[o,(b,ds)] = xm * w_syn.  The
            # broadcast in1 + fp32 out hits ~190G elem/s on DVE (measured
            # 691ns) -- bf16 out or a plain-2D in1 both halve the rate.
            prod = prpool.tile([P, 2 * FH], F32, tag="prod", name=f"prod{t}")
            nc.vector.tensor_mul(
                prod[:].rearrange("p (b q) -> p b q", b=BC),
                ps_t[:].rearrange("p (b q) -> p b q", b=BC),
                wsyn_sb[:, t * DS:(t + 1) * DS].unsqueeze(1)
                .broadcast_to([P, BC, DS]),
            )
            return prod

        def gps_tree(t, prod):
            # s-reduce as a GpSimd pairwise tree (fp32, ~2.8us/tile): slow
            # engine, but it runs in parallel with DVE -- used only for the
            # earliest tiles whose results aren't needed until the batched
            # tail.
            pv = prod[:].rearrange("p (bd s) -> p bd s", s=S)
            gr1 = smpool.tile([P, BD * 8], F32, tag="gr1", name=f"gr1{t}")
            nc.gpsimd.tensor_add(
                gr1[:].rearrange("p (bd s) -> p bd s", s=8),
                pv[:, :, 0:8], pv[:, :, 8:16],
            )
            g1v = gr1[:].rearrange("p (bd s) -> p bd s", s=8)
            gr2 = smpool.tile([P, BD * 4], F32, tag="gr2", name=f"gr2{t}")
            nc.gpsimd.tensor_add(
                gr2[:].rearrange("p (bd s) -> p bd s", s=4),
                g1v[:, :, 0:4], g1v[:, :, 4:8],
            )
            g2v = gr2[:].rearrange("p (bd s) -> p bd s", s=4)
            gr3 = smpool.tile([P, BD * 2], F32, tag="gr3", name=f"gr3{t}")
            nc.gpsimd.tensor_add(
                gr3[:].rearrange("p (bd s) -> p bd s", s=2),
                g2v[:, :, 0:2], g2v[:, :, 2:4],
            )
            g3v = gr3[:].rearrange("p (bd s) -> p bd s", s=2)
            nc.gpsimd.tensor_add(
                dp_all[:, t * BD:(t + 1) * BD].unsqueeze(2),
                g3v[:, :, 0:1], g3v[:, :, 1:2],
            )

        def dve_reduce(t, prod):
            # s-reduce on DVE (~1.22us/tile) into dp_all's (t, b, d) slot
            # (or dp7 for the last tile's private chain).
            dst = dp_all[:, t * BD:(t + 1) * BD]
            nc.vector.tensor_reduce(
                dst, prod[:].rearrange("p (bd s) -> p bd s", s=S),
                axis=AX.X, op=OP.add,
            )

        # --- Wave 1: o-tiles 0..3, kp-outer (paces with the DMA stream);
        # each tile's drain follows its last matmul, freeing 2 banks. ---
        pst = {}
        for t in range(GRP):
            pst[t] = pspool.tile([P, 2 * FH], F32, tag="ps", name=f"ps{t}")
        for kp in range(KP):
            for t in range(GRP):
                for h in range(2):
                    dr_mm(pst[t], t, kp, h)
        NGPS = 3   # tiles whose s-reduce runs as a GpSimd tree
        prods = {}
        for t in range(GRP):
            prods[t] = drain(t, pst[t])
            if t < NGPS:
                gps_tree(t, prods[t])
            else:
                dve_reduce(t, prods[t])

        # --- Wave 2: o-tiles 4..7 kp-inner, each claiming banks freed by
        # the corresponding wave-1 drain; drain ASAP after the 8th MM. ---
        for t in range(GRP, OT):
            ps_t = pspool.tile([P, 2 * FH], F32, tag="ps", name=f"ps{t}")
            for h in range(2):
                for kp in range(KP):
                    dr_mm(ps_t, t, kp, h)
            prods[t] = drain(t, ps_t)
            if t < OT - 1:
                dve_reduce(t, prods[t])

        # --- t7 private latency chain (DVE + ACT) -> out_sb[:, 56:64]. ---
        t7 = OT - 1
        dp7 = smpool.tile([P, BD], F32, tag="dp7", name="dp7")
        nc.vector.tensor_reduce(
            dp7[:], prods[t7][:].rearrange("p (bd s) -> p bd s", s=S),
            axis=AX.X, op=OP.add,
        )
        nc.vector.tensor_add(
            dp7[:].rearrange("p (b d) -> p b d", d=D),
            dp7[:].rearrange("p (b d) -> p b d", d=D),
            prm_sb[:, B0 + t7 * D:B0 + (t7 + 1) * D].unsqueeze(1)
            .broadcast_to([P, BC, D]),
        )
        dend7 = smpool.tile([P, BD], F32, tag="dend7", name="dend7")
        nc.scalar.activation(dend7[:], dp7[:], AF.Tanh)
        sp7 = smpool.tile([P, BD], F32, tag="sp7", name="sp7")
        nc.vector.tensor_mul(
            sp7[:].rearrange("p (b d) -> p b d", d=D),
            dend7[:].rearrange("p (b d) -> p b d", d=D),
            prm_sb[:, W1 + t7 * D:W1 + (t7 + 1) * D].unsqueeze(1)
            .broadcast_to([P, BC, D]),
        )
        soma7 = smpool.tile([P, BC], F32, tag="soma7", name="soma7")
        nc.vector.tensor_reduce(
            soma7[:], sp7[:].rearrange("p (b d) -> p b d", d=D),
            axis=AX.X, op=OP.add,
        )
        nc.scalar.activation(
            out_sb[:, t7 * BC:(t7 + 1) * BC], soma7[:], AF.Sigmoid,
            bias=prm_sb[:, B1 + t7:B1 + t7 + 1],
        )

        # --- Batched t0-6 tail: one wide op per stage. ---
        # dp_all[p, (t,b,d)] += b_dend[p, (t,d)]  (broadcast over b)
        nc.gpsimd.tensor_add(
            dp_all[:].rearrange("p (t b d) -> p t b d", t=TB, b=BC),
            dp_all[:].rearrange("p (t b d) -> p t b d", t=TB, b=BC),
            prm_sb[:, B0:B0 + TB * D]
            .rearrange("p (t d) -> p t d", t=TB).unsqueeze(2)
            .broadcast_to([P, TB, BC, D]),
        )
        dend_all = wpool.tile([P, TB * BD], F32, tag="dendall", name="dend_all")
        nc.scalar.activation(dend_all[:], dp_all[:], AF.Tanh)
        sp_all = wpool.tile([P, TB * BD], F32, tag="spall", name="sp_all")
        nc.vector.tensor_mul(
            sp_all[:].rearrange("p (t b d) -> p t b d", t=TB, b=BC),
            dend_all[:].rearrange("p (t b d) -> p t b d", t=TB, b=BC),
            prm_sb[:, W1:W1 + TB * D]
            .rearrange("p (t d) -> p t d", t=TB).unsqueeze(2)
            .broadcast_to([P, TB, BC, D]),
        )
        soma_all = wpool.tile([P, TB * BC], F32, tag="somaall", name="soma_all")
        nc.vector.tensor_reduce(
            soma_all[:], sp_all[:].rearrange("p (tb d) -> p tb d", d=D),
            axis=AX.X, op=OP.add,
        )
        nc.vector.tensor_add(
            soma_all[:].rearrange("p (t b) -> p t b", t=TB),
            soma_all[:].rearrange("p (t b) -> p t b", t=TB),
            prm_sb[:, B1:B1 + TB].unsqueeze(2).broadcast_to([P, TB, BC]),
        )
        nc.scalar.activation(out_sb[:, 0:TB * BC], soma_all[:], AF.Sigmoid)

        # Output on the (input-idle-by-now) Sync HWDGE.
        nc.sync.dma_start(out[:, :], out_sb[:])

    if legalize:
        legalize_waits(nc)
    return nc


def get_nc():
    key = "fp8dr3"
    if key not in _NC_CACHE:
        _NC_CACHE[key] = build_nc()
    return _NC_CACHE[key]


def pack_kpairs(a):
    """[N, C] -> [KP*P, 2*C], row kp*128+p holding [r=0 | r=1] halves of
    the k-pair (source row i = kp*256 + r*128 + p)."""
    C = a.shape[1]
    return np.ascontiguousarray(
        a.reshape(KP, 2, P, C).transpose(0, 2, 1, 3).reshape(KP * P, 2 * C)
    )


def prepare_in_maps(x, matriz_conexao, w_syn, b_dend, w_dend, b_soma):
    x = np.asarray(x, dtype=np.float32)
    mt_np = pack_kpairs(
        np.ascontiguousarray(np.asarray(matriz_conexao, np.float32).T)
    ).astype(NP_FP8)
    ws = np.asarray(w_syn, np.float32).reshape(OT, P, DS).transpose(1, 0, 2) \
        .reshape(P, OT * DS).astype(NP_FP8)
    bd = np.asarray(b_dend, np.float32).reshape(OT, P, D).transpose(1, 0, 2).reshape(P, OT * D)
    wd = np.asarray(w_dend, np.float32).reshape(OT, P, D).transpose(1, 0, 2).reshape(P, OT * D)
    bs = np.asarray(b_soma, np.float32).reshape(OT, P).T
    prm_np = np.ascontiguousarray(
        np.concatenate([bd, wd, bs], axis=1).astype(np.float32))
    xt = x.transpose(1, 0, 2, 3).reshape(N, B, DS)
    in_maps = []
    for c in range(NCORES):
        xc_np = pack_kpairs(
            np.ascontiguousarray(
                xt[:, c * BC:(c + 1) * BC, :].reshape(N, BC * DS))
        ).astype(NP_FP8)
        in_maps.append({"mt": mt_np, "xc": xc_np,
                        "wsyn": np.ascontiguousarray(ws), "prm": prm_np})
    return in_maps


def assemble_output(results):
    outs = []
    for c in range(NCORES):
        oc = np.asarray(results[c]["out"])          # [P, OT*BC] = (oi, (t, b))
        outs.append(oc.reshape(P, OT, BC).transpose(2, 1, 0).reshape(BC, N))
    return np.ascontiguousarray(np.concatenate(outs, axis=0).astype(np.float32))


def kernel(x, matriz_conexao, w_syn, b_dend, w_dend, b_soma):
    from concourse.bass_utils import run_bass_kernel_spmd
    in_maps = prepare_in_maps(x, matriz_conexao, w_syn, b_dend, w_dend, b_soma)
    nc = get_nc()
    res = run_bass_kernel_spmd(nc, in_maps, list(range(NCORES)))
    return assemble_output(res.results)
